# revision 23
# baseline (speedup 1.0000x reference)
"""MoE routing kernel (2 experts, D=128 -> H=512 -> O=2) for 8 Trainium2 cores.

Strategy: route on the HOST, compute on the device in fp32r.

The routing decision (argmin over 2 prototype distances) reduces to a sign
test q = x.(p1-p0) vs a threshold, computed exactly on the host in float64.
The host then SORTS samples by expert, pads each expert segment to a
256-column multiple (pads replicate real samples), and uploads each core's
shard pre-transposed as xT [128d, N]. The device runs a pure dense
single-expert MLP over blocks of 512 (plus at most one 256 tail block per
expert, keeping f32r at its 1 col/cycle rate which needs ap>=256):

  per block (expert e fixed per block, L in {512, 256}):
    1. DMA xT block [128d, L] (f32r) HBM->SBUF
    2. PE layer-1: 4 f32r matmuls (w1_e tiles stationary, xT moving) -> PSUM
    3. ACT/DVE: fused relu(z + b1_e) -> h SBUF f32r
    4. PE layer-2: 4 f32r matmuls (w2_e [128,2] stationary, h moving),
       PSUM-accumulated -> out [2o, L]
    5. ACT/DVE copy PSUM->SBUF (alternating), DMA out transposed [2, N]

The loop is software-pipelined by one block: the PE interleaves L1 matmuls
of block i with L2 matmuls of block i-1, so every L1 LDWEIGHTS (~107ns,
not hideable between back-to-back f32r matmuls) hides under an L2 stream
whose own weight load is 2 columns.

Startup hiding: dependency-free dummy matmuls warm the PE clock gate
(1.2 -> 2.4 GHz after ~3.4us of activity); a dummy activation preloads the
ACT table; the first x blocks and w1 tiles are DMA'd in small chunks across
several queues AND several descriptor sequencers (sync/scalar/gpsimd), since
one queue moves only ~22.5 GB/s and one descriptor costs ~650ns of sequencer
time. The host adds b2 and inverse-permutes the output.
"""

import numpy as np

import concourse.bacc as bacc
import concourse.bass as bass
import concourse.mybir as mybir
import concourse.tile as tile
from concourse.bass_utils import run_bass_kernel_spmd

F32 = mybir.dt.float32
F32R = mybir.dt.float32r

N_CORES = 8
D = 128
H = 512
E = 2
O = 2
NJ = H // 128         # 4 k/h tiles of 128 per expert
BLK = 512             # max samples per block
QUANT = 256           # per-core expert segment quantum
N_WARMUP = 8          # dependency-free dummy matmuls to warm the PE clock


def _blocks(c0, c1):
    """Block schedule: (expert, col offset, length) with L in {512, 256}."""
    blocks = []
    off = 0
    for e, ce in ((0, c0), (1, c1)):
        n_full, tail = ce // BLK, ce % BLK
        for _ in range(n_full):
            blocks.append((e, off, BLK))
            off += BLK
        if tail:
            blocks.append((e, off, tail))
            off += tail
    return blocks


def _build_program(c0: int, c1: int):
    """Per-core program: c0 expert-0 columns then c1 expert-1 columns."""
    n_shard = c0 + c1

    nc = bacc.Bacc(
        "TRN2",
        target_bir_lowering=False,
        debug=False,
        enable_asserts=False,
        num_devices=1,
    )

    xt = nc.dram_tensor("xt", [128, n_shard], F32R, kind="ExternalInput").ap()
    w1t = nc.dram_tensor("w1t", [128, E * H], F32R, kind="ExternalInput").ap()
    b1c = nc.dram_tensor("b1c", [128, E * NJ], F32, kind="ExternalInput").ap()
    w2c = nc.dram_tensor("w2c", [128, E * NJ, O], F32R, kind="ExternalInput").ap()
    out = nc.dram_tensor("out", [O, n_shard], F32, kind="ExternalOutput").ap()

    with tile.TileContext(nc) as tc:
        _body(tc, c0, c1, xt, w1t, b1c, w2c, out)

    nc.compile()
    return nc


def _body(tc, c0, c1, xt, w1t, b1c, w2c, out):
    nc = tc.nc
    Relu = mybir.ActivationFunctionType.Relu
    Alu = mybir.AluOpType
    blocks = _blocks(c0, c1)
    nblk = len(blocks)

    with (
        tc.tile_pool(name="consts", bufs=1) as cpool,
        tc.tile_pool(name="xb", bufs=12) as xb_pool,
        tc.tile_pool(name="h", bufs=4) as h_pool,
        tc.tile_pool(name="osb", bufs=3) as o_pool,
        tc.tile_pool(name="zp", bufs=6, space="PSUM") as zp_pool,
        tc.tile_pool(name="op", bufs=2, space="PSUM") as op_pool,
    ):
        # PE warmup on a memset scratch tile: no DMA deps, so the tensor
        # engine is busy from t~0 and the HAM clock gate is warm (2.4 GHz)
        # when the first real matmul issues.
        junk = cpool.tile([128, BLK], mybir.dt.bfloat16)
        nc.vector.memset(junk[:], 0.0)
        warm = zp_pool.tile([128, BLK], F32, tag="zp")
        for _ in range(N_WARMUP):
            nc.tensor.matmul(
                warm[:], lhsT=junk[:, 0:128], rhs=junk[:], start=True, stop=True
            )
        # Preload the ACT function table during the DMA wait.
        scr = cpool.tile([1, 8], F32)
        nc.scalar.activation(scr[:], junk[0:1, 0:8], Relu, scale=1.0)

        engs = [nc.sync, nc.gpsimd, nc.scalar]

        # First two x blocks land first, in 8 x 32KB chunks across queues
        # and sequencers, so block 0 gates the pipeline minimally.
        early_xb = []
        for bi in range(min(2, nblk)):
            _, off, L = blocks[bi]
            xb = xb_pool.tile([128, BLK], F32R, tag="xb")
            nsp = 8 if bi == 0 else 4
            cw = L // nsp
            for i in range(nsp):
                engs[i % 3].dma_start(
                    xb[:, i * cw : (i + 1) * cw],
                    xt[:, off + i * cw : off + (i + 1) * cw],
                )
            early_xb.append(xb)

        # Constants; the first w1 tile (needed by the first real matmul) is
        # partition-split in two for minimum latency.
        w1t_sb = cpool.tile([128, E * H], F32R)
        nc.sync.dma_start(w1t_sb[0:64, 0:128], w1t[0:64, 0:128])
        nc.gpsimd.dma_start(w1t_sb[64:128, 0:128], w1t[64:128, 0:128])
        for i in range(1, 8):
            engs[i % 3].dma_start(
                w1t_sb[:, i * 128 : (i + 1) * 128], w1t[:, i * 128 : (i + 1) * 128]
            )
        b1c_sb = cpool.tile([128, E * NJ], F32)
        nc.sync.dma_start(b1c_sb[:], b1c)
        w2c_sb = cpool.tile([128, E * NJ, O], F32R)
        nc.gpsimd.dma_start(w2c_sb[:], w2c)

        prev = None  # (h tile, expert, offset, length, parity)

        def _emit_l2(h, e, off, L, par):
            op_ps = op_pool.tile([O, BLK], F32, tag="op")
            for j in range(NJ):
                nc.tensor.matmul(
                    op_ps[:, 0:L],
                    lhsT=w2c_sb[:, e * NJ + j, :],
                    rhs=h[:, j, 0:L],
                    start=(j == 0),
                    stop=(j == NJ - 1),
                )
            osb = o_pool.tile([O, BLK], F32, tag="osb")
            if par == 0:
                nc.scalar.copy(osb[:, 0:L], op_ps[:, 0:L])
            else:
                nc.vector.tensor_copy(osb[:, 0:L], op_ps[:, 0:L])
            nc.sync.dma_start(out[:, off : off + L], osb[:, 0:L])

        for bi in range(nblk):
            e, off, L = blocks[bi]

            if bi < len(early_xb):
                xb = early_xb[bi]
            else:
                # steady state: two DMA queues per block, descriptors from
                # two different sequencers
                xb = xb_pool.tile([128, BLK], F32R, tag="xb")
                hb = L // 2
                nc.sync.dma_start(xb[:, 0:hb], xt[:, off : off + hb])
                nc.gpsimd.dma_start(xb[:, hb:L], xt[:, off + hb : off + L])

            # layer 1 (interleaved on PE with layer 2 of block bi-1) + relu
            h = h_pool.tile([128, NJ, BLK], F32R)
            op_prev = None
            if prev is not None:
                ph, pe, poff, pL, ppar = prev
                op_prev = op_pool.tile([O, BLK], F32, tag="op")
            for j in range(NJ):
                zp = zp_pool.tile([128, BLK], F32, tag="zp")
                nc.tensor.matmul(
                    zp[:, 0:L],
                    lhsT=w1t_sb[:, (e * H + j * 128) : (e * H + (j + 1) * 128)],
                    rhs=xb[:, 0:L],
                    start=True,
                    stop=True,
                )
                if op_prev is not None:
                    nc.tensor.matmul(
                        op_prev[:, 0:pL],
                        lhsT=w2c_sb[:, pe * NJ + j, :],
                        rhs=ph[:, j, 0:pL],
                        start=(j == 0),
                        stop=(j == NJ - 1),
                    )
                jj = e * NJ + j
                if j % 2 == 0:
                    nc.scalar.activation(
                        h[:, j, 0:L], zp[:, 0:L], Relu,
                        bias=b1c_sb[:, jj : jj + 1], scale=1.0,
                    )
                else:
                    nc.vector.tensor_scalar(
                        out=h[:, j, 0:L],
                        in0=zp[:, 0:L],
                        scalar1=b1c_sb[:, jj : jj + 1],
                        scalar2=0.0,
                        op0=Alu.add,
                        op1=Alu.max,
                    )
            if op_prev is not None:
                osb = o_pool.tile([O, BLK], F32, tag="osb")
                if ppar == 0:
                    nc.scalar.copy(osb[:, 0:pL], op_prev[:, 0:pL])
                else:
                    nc.vector.tensor_copy(osb[:, 0:pL], op_prev[:, 0:pL])
                nc.sync.dma_start(out[:, poff : poff + pL], osb[:, 0:pL])
            prev = (h, e, off, L, bi % 2)

        # epilogue: layer 2 of the final block
        ph, pe, poff, pL, _ = prev
        _emit_l2(ph, pe, poff, pL, nblk % 2)


def _pack_consts(w1, b1, w2):
    w1 = np.asarray(w1, np.float32)
    b1 = np.asarray(b1, np.float32)
    w2 = np.asarray(w2, np.float32)
    # w1t[d, e*H + h] = w1[e, h, d]
    w1t = np.ascontiguousarray(np.transpose(w1, (2, 0, 1)).reshape(D, E * H))
    # b1c[p, e*NJ + j] = b1[e, j*128 + p]
    b1c = np.ascontiguousarray(
        b1.reshape(E, NJ, 128).transpose(2, 0, 1).reshape(128, E * NJ)
    )
    # w2c[p, e*NJ + j, o] = w2[e, o, j*128 + p]
    w2c = np.ascontiguousarray(
        w2.reshape(E, O, NJ, 128).transpose(3, 0, 2, 1).reshape(128, E * NJ, O)
    )
    return dict(w1t=w1t, b1c=b1c, w2c=w2c)


_PROG_CACHE = {}


def _get_program(c0, c1):
    key = (c0, c1)
    if key not in _PROG_CACHE:
        _PROG_CACHE[key] = _build_program(c0, c1)
    return _PROG_CACHE[key]


def _pad_chunk(idx, target):
    """Pad index chunk to target length by repeating the last index."""
    if len(idx) == target:
        return idx
    pad = np.full(target - len(idx), idx[-1] if len(idx) else 0, dtype=np.int64)
    return np.concatenate([np.asarray(idx, np.int64), pad])


def kernel(x, w1, b1, w2, b2, prototypes, _trace=False):
    x = np.ascontiguousarray(np.asarray(x, np.float32))
    b2 = np.asarray(b2, np.float32)
    p = np.asarray(prototypes, np.float64)
    btot = x.shape[0]

    # --- host routing (exact, float64) ---
    rvec = p[1] - p[0]
    thr = (p[1] @ p[1] - p[0] @ p[0]) / 2.0
    q = x.astype(np.float64) @ rvec
    m1 = q > thr                       # expert 1 wins (ties -> expert 0)
    idx0 = np.flatnonzero(~m1)
    idx1 = np.flatnonzero(m1)

    chunks0 = np.array_split(idx0, N_CORES)
    chunks1 = np.array_split(idx1, N_CORES)
    mx0 = max(len(c) for c in chunks0)
    mx1 = max(len(c) for c in chunks1)
    c0 = max(QUANT, QUANT * -(-mx0 // QUANT))   # per-core expert-0 columns
    c1 = max(QUANT, QUANT * -(-mx1 // QUANT))

    nc = _get_program(c0, c1)
    consts = _pack_consts(w1, b1, w2)

    in_maps = []
    for c in range(N_CORES):
        ic = np.concatenate(
            [_pad_chunk(chunks0[c], c0), _pad_chunk(chunks1[c], c1)]
        )
        m = dict(consts)
        m["xt"] = np.ascontiguousarray(x[ic].T)
        in_maps.append(m)

    res = run_bass_kernel_spmd(
        nc, in_maps, core_ids=list(range(N_CORES)), trace=_trace
    )

    full = np.empty((btot, O), np.float32)
    for c in range(N_CORES):
        y = np.asarray(res.results[c]["out"])     # [O, N]
        yt = np.ascontiguousarray(y.T)            # [N, O]
        yt[:c0] += b2[0]
        yt[c0:] += b2[1]
        n0c, n1c = len(chunks0[c]), len(chunks1[c])
        full[chunks0[c]] = yt[:n0c]               # pads never scattered
        full[chunks1[c]] = yt[c0 : c0 + n1c]
    if _trace:
        return full, res
    return full


# revision 24
# speedup vs baseline: 1.1365x; 1.1365x over previous
"""MoE routing kernel (2 experts, D=128 -> H=512 -> O=2) for 8 Trainium2 cores.

Strategy: route on the HOST, compute on the device in fp32r.

The routing decision (argmin over 2 prototype distances) reduces to a sign
test q = x.(p1-p0) vs a threshold, computed exactly on the host in float64.
The host then SORTS samples by expert, pads each expert segment to a
256-column multiple (pads replicate real samples), and uploads each core's
shard pre-transposed as xT [128d, N]. The device runs a pure dense
single-expert MLP over blocks of 512 (plus at most one 256 tail block per
expert, keeping f32r at its 1 col/cycle rate which needs ap>=256):

  per block (expert e fixed per block, L in {512, 256}):
    1. DMA xT block [128d, L] (f32r) HBM->SBUF
    2. PE layer-1: 4 f32r matmuls (w1_e tiles stationary, xT moving) -> PSUM
    3. ACT/DVE: fused relu(z + b1_e) -> h SBUF f32r
    4. PE layer-2: 4 f32r matmuls (w2_e [128,2] stationary, h moving),
       PSUM-accumulated -> out [2o, L]
    5. ACT/DVE copy PSUM->SBUF (alternating), DMA out transposed [2, N]

The loop is software-pipelined by one block: the PE interleaves L1 matmuls
of block i with L2 matmuls of block i-1, so every L1 LDWEIGHTS (~107ns,
not hideable between back-to-back f32r matmuls) hides under an L2 stream
whose own weight load is 2 columns.

Startup hiding: dependency-free dummy matmuls warm the PE clock gate
(1.2 -> 2.4 GHz after ~3.4us of activity); a dummy activation preloads the
ACT table; the first x blocks and w1 tiles are DMA'd in small chunks across
several queues AND several descriptor sequencers (sync/scalar/gpsimd), since
one queue moves only ~22.5 GB/s and one descriptor costs ~650ns of sequencer
time. The host adds b2 and inverse-permutes the output.
"""

import numpy as np

import concourse.bacc as bacc
import concourse.bass as bass
import concourse.mybir as mybir
import concourse.tile as tile
from concourse.bass_utils import run_bass_kernel_spmd

F32 = mybir.dt.float32
F32R = mybir.dt.float32r

N_CORES = 8
D = 128
H = 512
E = 2
O = 2
NJ = H // 128         # 4 k/h tiles of 128 per expert
BLK = 512             # max samples per block
QUANT = 256           # per-core expert segment quantum
N_WARMUP = 8          # dependency-free dummy matmuls to warm the PE clock


def _blocks(c0, c1):
    """Block schedule: (expert, col offset, length) with L in {512, 256}."""
    blocks = []
    off = 0
    for e, ce in ((0, c0), (1, c1)):
        n_full, tail = ce // BLK, ce % BLK
        for _ in range(n_full):
            blocks.append((e, off, BLK))
            off += BLK
        if tail:
            blocks.append((e, off, tail))
            off += tail
    return blocks


def _build_program(c0: int, c1: int):
    """Per-core program: c0 expert-0 columns then c1 expert-1 columns."""
    n_shard = c0 + c1

    nc = bacc.Bacc(
        "TRN2",
        target_bir_lowering=False,
        debug=False,
        enable_asserts=False,
        num_devices=1,
    )

    xt = nc.dram_tensor("xt", [128, n_shard], F32R, kind="ExternalInput").ap()
    w1t = nc.dram_tensor("w1t", [128, E * H], F32R, kind="ExternalInput").ap()
    b1c = nc.dram_tensor("b1c", [128, E * NJ], F32, kind="ExternalInput").ap()
    w2c = nc.dram_tensor("w2c", [128, E * NJ, O], F32R, kind="ExternalInput").ap()
    out = nc.dram_tensor("out", [O, n_shard], F32, kind="ExternalOutput").ap()

    with tile.TileContext(nc) as tc:
        _body(tc, c0, c1, xt, w1t, b1c, w2c, out)

    nc.compile()
    return nc


def _body(tc, c0, c1, xt, w1t, b1c, w2c, out):
    nc = tc.nc
    Relu = mybir.ActivationFunctionType.Relu
    Alu = mybir.AluOpType
    blocks = _blocks(c0, c1)
    nblk = len(blocks)

    with (
        tc.tile_pool(name="consts", bufs=1) as cpool,
        tc.tile_pool(name="xb", bufs=10) as xb_pool,
        tc.tile_pool(name="h", bufs=3) as h_pool,
        tc.tile_pool(name="osb", bufs=3) as o_pool,
        tc.tile_pool(name="zp", bufs=4, space="PSUM") as zp_pool,
        tc.tile_pool(name="op", bufs=2, space="PSUM") as op_pool,
    ):
        # PE warmup on a memset scratch tile: no DMA deps, so the tensor
        # engine is busy from t~0 and the HAM clock gate is warm (2.4 GHz)
        # when the first real matmul issues.
        junk = cpool.tile([128, BLK], mybir.dt.bfloat16)
        nc.vector.memset(junk[:], 0.0)
        warm = zp_pool.tile([128, BLK], F32, tag="zp")
        for _ in range(N_WARMUP):
            nc.tensor.matmul(
                warm[:], lhsT=junk[:, 0:128], rhs=junk[:], start=True, stop=True
            )
        # Preload the ACT function table during the DMA wait.
        scr = cpool.tile([1, 8], F32)
        nc.scalar.activation(scr[:], junk[0:1, 0:8], Relu, scale=1.0)

        engs = [nc.sync, nc.gpsimd, nc.scalar]

        # First two x blocks land first, in 8 x 32KB chunks across queues
        # and sequencers, so block 0 gates the pipeline minimally.
        early_xb = []
        for bi in range(min(2, nblk)):
            _, off, L = blocks[bi]
            xb = xb_pool.tile([128, BLK], F32R, tag="xb")
            nsp = 8 if bi == 0 else 4
            cw = L // nsp
            for i in range(nsp):
                engs[i % 3].dma_start(
                    xb[:, i * cw : (i + 1) * cw],
                    xt[:, off + i * cw : off + (i + 1) * cw],
                )
            early_xb.append(xb)

        # Constants; the first w1 tile (needed by the first real matmul) is
        # partition-split in two for minimum latency.
        w1t_sb = cpool.tile([128, E * H], F32R)
        nc.sync.dma_start(w1t_sb[0:64, 0:128], w1t[0:64, 0:128])
        nc.gpsimd.dma_start(w1t_sb[64:128, 0:128], w1t[64:128, 0:128])
        for i in range(1, 8):
            engs[i % 3].dma_start(
                w1t_sb[:, i * 128 : (i + 1) * 128], w1t[:, i * 128 : (i + 1) * 128]
            )
        b1c_sb = cpool.tile([128, E * NJ], F32)
        nc.sync.dma_start(b1c_sb[:], b1c)
        w2c_sb = cpool.tile([128, E * NJ, O], F32R)
        nc.gpsimd.dma_start(w2c_sb[:], w2c)

        prev = None  # (h tile, expert, offset, length, parity)

        def _emit_l2(h, e, off, L, par):
            op_ps = op_pool.tile([O, BLK], F32, tag="op")
            for j in range(NJ):
                nc.tensor.matmul(
                    op_ps[:, 0:L],
                    lhsT=w2c_sb[:, e * NJ + j, :],
                    rhs=h[:, j, 0:L],
                    start=(j == 0),
                    stop=(j == NJ - 1),
                )
            osb = o_pool.tile([O, BLK], F32, tag="osb")
            if par == 0:
                nc.scalar.copy(osb[:, 0:L], op_ps[:, 0:L])
            else:
                nc.vector.tensor_copy(osb[:, 0:L], op_ps[:, 0:L])
            nc.sync.dma_start(out[:, off : off + L], osb[:, 0:L])

        for bi in range(nblk):
            e, off, L = blocks[bi]

            if bi < len(early_xb):
                xb = early_xb[bi]
            else:
                # steady state: two DMA queues per block, descriptors from
                # two different sequencers
                xb = xb_pool.tile([128, BLK], F32R, tag="xb")
                hb = L // 2
                nc.sync.dma_start(xb[:, 0:hb], xt[:, off : off + hb])
                nc.gpsimd.dma_start(xb[:, hb:L], xt[:, off + hb : off + L])

            # layer 1 (interleaved on PE with layer 2 of block bi-1) + relu
            h = h_pool.tile([128, NJ, BLK], F32R)
            op_prev = None
            if prev is not None:
                ph, pe, poff, pL, ppar = prev
                op_prev = op_pool.tile([O, BLK], F32, tag="op")
            for j in range(NJ):
                zp = zp_pool.tile([128, BLK], F32, tag="zp")
                nc.tensor.matmul(
                    zp[:, 0:L],
                    lhsT=w1t_sb[:, (e * H + j * 128) : (e * H + (j + 1) * 128)],
                    rhs=xb[:, 0:L],
                    start=True,
                    stop=True,
                )
                if op_prev is not None:
                    nc.tensor.matmul(
                        op_prev[:, 0:pL],
                        lhsT=w2c_sb[:, pe * NJ + j, :],
                        rhs=ph[:, j, 0:pL],
                        start=(j == 0),
                        stop=(j == NJ - 1),
                    )
                jj = e * NJ + j
                if j % 2 == 0:
                    nc.scalar.activation(
                        h[:, j, 0:L], zp[:, 0:L], Relu,
                        bias=b1c_sb[:, jj : jj + 1], scale=1.0,
                    )
                else:
                    nc.vector.tensor_scalar(
                        out=h[:, j, 0:L],
                        in0=zp[:, 0:L],
                        scalar1=b1c_sb[:, jj : jj + 1],
                        scalar2=0.0,
                        op0=Alu.add,
                        op1=Alu.max,
                    )
            if op_prev is not None:
                osb = o_pool.tile([O, BLK], F32, tag="osb")
                if ppar == 0:
                    nc.scalar.copy(osb[:, 0:pL], op_prev[:, 0:pL])
                else:
                    nc.vector.tensor_copy(osb[:, 0:pL], op_prev[:, 0:pL])
                nc.sync.dma_start(out[:, poff : poff + pL], osb[:, 0:pL])
            prev = (h, e, off, L, bi % 2)

        # epilogue: layer 2 of the final block
        ph, pe, poff, pL, _ = prev
        _emit_l2(ph, pe, poff, pL, nblk % 2)


def _pack_consts(w1, b1, w2):
    w1 = np.asarray(w1, np.float32)
    b1 = np.asarray(b1, np.float32)
    w2 = np.asarray(w2, np.float32)
    # w1t[d, e*H + h] = w1[e, h, d]
    w1t = np.ascontiguousarray(np.transpose(w1, (2, 0, 1)).reshape(D, E * H))
    # b1c[p, e*NJ + j] = b1[e, j*128 + p]
    b1c = np.ascontiguousarray(
        b1.reshape(E, NJ, 128).transpose(2, 0, 1).reshape(128, E * NJ)
    )
    # w2c[p, e*NJ + j, o] = w2[e, o, j*128 + p]
    w2c = np.ascontiguousarray(
        w2.reshape(E, O, NJ, 128).transpose(3, 0, 2, 1).reshape(128, E * NJ, O)
    )
    return dict(w1t=w1t, b1c=b1c, w2c=w2c)


_PROG_CACHE = {}


def _get_program(c0, c1):
    key = (c0, c1)
    if key not in _PROG_CACHE:
        _PROG_CACHE[key] = _build_program(c0, c1)
    return _PROG_CACHE[key]


def _pad_chunk(idx, target):
    """Pad index chunk to target length by repeating the last index."""
    if len(idx) == target:
        return idx
    pad = np.full(target - len(idx), idx[-1] if len(idx) else 0, dtype=np.int64)
    return np.concatenate([np.asarray(idx, np.int64), pad])


def kernel(x, w1, b1, w2, b2, prototypes, _trace=False):
    x = np.ascontiguousarray(np.asarray(x, np.float32))
    b2 = np.asarray(b2, np.float32)
    p = np.asarray(prototypes, np.float64)
    btot = x.shape[0]

    # --- host routing (exact, float64) ---
    rvec = p[1] - p[0]
    thr = (p[1] @ p[1] - p[0] @ p[0]) / 2.0
    q = x.astype(np.float64) @ rvec
    m1 = q > thr                       # expert 1 wins (ties -> expert 0)
    idx0 = np.flatnonzero(~m1)
    idx1 = np.flatnonzero(m1)

    chunks0 = np.array_split(idx0, N_CORES)
    chunks1 = np.array_split(idx1, N_CORES)
    mx0 = max(len(c) for c in chunks0)
    mx1 = max(len(c) for c in chunks1)
    c0 = max(QUANT, QUANT * -(-mx0 // QUANT))   # per-core expert-0 columns
    c1 = max(QUANT, QUANT * -(-mx1 // QUANT))

    nc = _get_program(c0, c1)
    consts = _pack_consts(w1, b1, w2)

    in_maps = []
    for c in range(N_CORES):
        ic = np.concatenate(
            [_pad_chunk(chunks0[c], c0), _pad_chunk(chunks1[c], c1)]
        )
        m = dict(consts)
        m["xt"] = np.ascontiguousarray(x[ic].T)
        in_maps.append(m)

    res = run_bass_kernel_spmd(
        nc, in_maps, core_ids=list(range(N_CORES)), trace=_trace
    )

    full = np.empty((btot, O), np.float32)
    for c in range(N_CORES):
        y = np.asarray(res.results[c]["out"])     # [O, N]
        yt = np.ascontiguousarray(y.T)            # [N, O]
        yt[:c0] += b2[0]
        yt[c0:] += b2[1]
        n0c, n1c = len(chunks0[c]), len(chunks1[c])
        full[chunks0[c]] = yt[:n0c]               # pads never scattered
        full[chunks1[c]] = yt[c0 : c0 + n1c]
    if _trace:
        return full, res
    return full
